# revision 1
# baseline (speedup 1.0000x reference)
"""AdaptGNN 3-layer message passing on 8 TRN2 NeuronCores.

Data-parallel over batch B=8: core c owns batch element c. Per core:
  h = x                                         [N=2048, D=128]
  for l in 0..2:
      hW   = h @ Wl + bl                        [N, 128]
      inv  = 1 / max(||hW||_row, eps)
      cos  = (inv inv^T) * (hW hW^T)            [N, N]
      h    = (ew * cos) @ hW                    [N, N] @ [N, 128]
      if l < 2: h = relu(h)

On-chip formulation (layouts chosen so no transposes of h are needed
beyond 16 PE tile-transposes per layer):
  - Loop state is hT_raw [128, N] (bf16, d on partitions) plus a deferred
    per-node scale s[n] (h_true[n,:] = s[n] * hT_raw[:,n]).
  - hW_nat tile (n-block t): matmul(lhsT=hT[:, t], rhs=W) -> psum, then
    scalar_tensor_tensor: (psum * s[t]) + b_bcast -> bf16. True hW values.
  - hWT via PE tile transpose of hW_nat.
  - Gram G[q,p] = hWT[:,q].T @ hWT[:,p] in psum; then
    MT[q,p] = ew[p,q] * inv[q] * G[q,p] via either
      (a) one DVE scalar_tensor_tensor (psum*inv)*ewT, or
      (b) ACT scaled-copy psum->sbuf bf16, then DVE/GPSIMD multiply,
    distributed to balance engine load.
  - aggT'[c,p] += hW_nat[q].T @ MT[q,p] accumulated over q in psum
    (= agg[p,c] / inv[p]; the missing inv[p] is the next layer's s).
  - next hT_raw = relu(aggT') (relu commutes with positive scale).
  - Final layer: transpose aggT' tiles to natural, multiply by inv[p]
    (per-partition), DMA out in f32.
  - ewT (transposed edge weights, bf16) built once: one contiguous SWDGE
    cast-DMA f32->bf16 into DRAM scratch (issued first so the 16MiB read
    starts immediately), then 16 HWDGE xbar transpose-DMAs into resident
    SBUF (64KB/partition). The xbar quiesces against all in-flight DMAs,
    so cast and transposes are inherently serial (~85us).
"""

import functools

import numpy as np

N = 2048
D = 128
T = N // 128          # 16 row blocks
NCHUNK = N // 512     # 4 free-dim chunks for N=512 matmuls
N_CORES = 8
EPS = 1e-12


@functools.lru_cache(maxsize=1)
def build_nc():
    import concourse.bass as bass
    from concourse import bacc, masks, mybir, tile

    f32 = mybir.dt.float32
    bf16 = mybir.dt.bfloat16
    AF = mybir.ActivationFunctionType
    ALU = mybir.AluOpType

    nc = bacc.Bacc(None, target_bir_lowering=False)

    x_d = nc.declare_dram_parameter("x", [N, D], f32, isOutput=False)
    ew_d = nc.declare_dram_parameter("edge_weight", [N, N], f32, isOutput=False)
    w_d = []
    b_d = []
    for l in range(3):
        w_d.append(nc.declare_dram_parameter(f"W{l}", [D, D], f32, isOutput=False))
        b_d.append(nc.declare_dram_parameter(f"b{l}", [D], f32, isOutput=False))
    out_d = nc.declare_dram_parameter("out", [N, D], f32, isOutput=True)

    with tile.TileContext(nc) as tc:
        with (
            tc.tile_pool(name="persist", bufs=1) as persist,
            tc.tile_pool(name="consts", bufs=1) as consts,
            tc.tile_pool(name="hts", bufs=2) as hts,
            tc.tile_pool(name="hwn_p", bufs=2) as hwn_p,
            tc.tile_pool(name="hwt_p", bufs=2) as hwt_p,
            tc.tile_pool(name="inv_p", bufs=2) as inv_p,
            tc.tile_pool(name="scr_p", bufs=1) as scr_p,
            tc.tile_pool(name="gs_p", bufs=6) as gs_p,
            tc.tile_pool(name="mt_p", bufs=16) as mt_p,
            tc.tile_pool(name="lw_p", bufs=2) as lw_p,
            tc.tile_pool(name="psum", bufs=4, space="PSUM") as psum,
            tc.tile_pool(name="dram", bufs=4, space="DRAM") as dram,
        ):
            # ---- W casts (tiny, ahead of the big ew cast on SWDGE) ----
            W_bf = []
            for l in range(3):
                wb = consts.tile([128, 128], bf16, tag=f"wbf{l}", name=f"wb{l}")
                nc.gpsimd.dma_start(wb[:], w_d[l][:, :])  # cast f32->bf16
                W_bf.append(wb)

            # ---- edge_weight cast: start the 16MiB read ASAP ----
            ewT = persist.tile([128, T, N], bf16, tag="ewT")  # slab qb at [:, qb, :]
            ewbf = dram.tile([N, N], bf16, tag="ewbf", name="ewbf")
            nc.gpsimd.dma_start(ewbf[:], ew_d[:, :])  # one contiguous 16MiB cast

            # ---- constants ----
            ident_f32 = consts.tile([128, 128], f32, tag="idf")
            ident_bf = consts.tile([128, 128], bf16, tag="idb")
            masks.make_identity(nc, ident_f32[:])
            masks.make_identity(nc, ident_bf[:])
            ones_row = consts.tile([1, 128], f32, tag="ones")
            nc.vector.memset(ones_row[:], 1.0)

            # ---- biases, x ----
            b_bc = []
            for l in range(3):
                brow = lw_p.tile([1, 128], f32, tag="brow", name=f"brow{l}")
                nc.sync.dma_start(brow[:], b_d[l].ap().rearrange("(o d) -> o d", o=1))
                bb = consts.tile([128, 128], f32, tag=f"bbc{l}", name=f"bb{l}")
                ps = psum.tile([128, 128], f32, tag="g", bufs=4)
                nc.tensor.matmul(ps[:], ones_row[:], brow[:])
                nc.scalar.activation(bb[:], ps[:], AF.Copy)
                b_bc.append(bb)

            # x natural (f32, via HWDGE so the SWDGE queue is free for ew)
            xn = persist.tile([128, T, 128], f32, tag="xn")
            nc.sync.dma_start(
                xn[:], x_d.ap().rearrange("(t p) d -> p t d", p=128)
            )

            # ew transposes (xbar quiesces against in-flight DMAs; these run
            # after the cast completes, in qb order for the L0 q-loop)
            for qb in range(T):
                nc.sync.dma_start(
                    ewT[:, qb, :], ewbf[:, qb * 128:(qb + 1) * 128],
                    transpose=True,
                )

            # x transpose -> hT0 (f32 transpose, cast to bf16 on evac)
            hT = hts.tile([128, N], bf16, tag="hT")
            for t in range(T):
                ps = psum.tile([128, 128], f32, tag="g", bufs=4)
                nc.tensor.transpose(ps[:], xn[:, t, :], ident_f32[:])
                nc.scalar.activation(hT[:, t * 128:(t + 1) * 128], ps[:], AF.Copy)

            # ---- layers ----
            for l in range(3):
                hwn = hwn_p.tile([128, T, 128], bf16, tag="hwn")
                nhT = hwt_p.tile([128, N], bf16, tag="nhT")
                n2 = inv_p.tile([128, T], f32, tag="n2")
                inv = inv_p.tile([128, T], f32, tag="inv")
                sq_scr = scr_p.tile([128, 128], f32, tag="sq")

                for t in range(T):
                    ps = psum.tile([128, 128], f32, tag="g", bufs=4)
                    nc.tensor.matmul(
                        ps[:], hT[:, t * 128:(t + 1) * 128], W_bf[l][:]
                    )
                    nc.vector.tensor_add(hwn[:, t, :], ps[:], b_bc[l][:])
                    nc.vector.tensor_mul(sq_scr[:], hwn[:, t, :], hwn[:, t, :])
                    nc.vector.reduce_sum(
                        n2[:, t:t + 1], sq_scr[:], axis=mybir.AxisListType.X
                    )

                # inv = 1 / max(sqrt(n2), eps)
                nrm = inv_p.tile([128, T], f32, tag="nrm")
                nc.scalar.activation(nrm[:], n2[:], AF.Sqrt)
                nc.vector.tensor_scalar_max(nrm[:], nrm[:], EPS)
                nc.vector.reciprocal(inv[:], nrm[:])

                # nh (normalized) tiles -> transpose -> nhT
                for t in range(T):
                    nh = scr_p.tile([128, 128], bf16, tag="nh", bufs=3)
                    nc.vector.tensor_scalar_mul(
                        nh[:], hwn[:, t, :], inv[:, t:t + 1]
                    )
                    ps2 = psum.tile([128, 128], bf16, tag="g", bufs=4)
                    nc.tensor.transpose(ps2[:], nh[:], ident_bf[:])
                    nc.scalar.activation(
                        nhT[:, t * 128:(t + 1) * 128], ps2[:], AF.Copy
                    )

                agg = [
                    psum.tile([128, 512], f32, tag="agg", name=f"agg{l}_{j}")
                    for j in range(NCHUNK)
                ]
                for qb in range(T):
                    for j in range(NCHUNK):
                        g_ps = psum.tile([128, 512], f32, tag="g", bufs=4)
                        nc.tensor.matmul(
                            g_ps[:],
                            nhT[:, qb * 128:(qb + 1) * 128],
                            nhT[:, j * 512:(j + 1) * 512],
                        )
                        ew_sl = ewT[:, qb, j * 512:(j + 1) * 512]
                        mt = mt_p.tile([128, 512], bf16, tag="mt")
                        if l > 0 and j == 0:
                            # fused: cos(psum) * ewT in one DVE op
                            nc.vector.tensor_tensor(
                                mt[:], g_ps[:], ew_sl, op=ALU.mult
                            )
                        else:
                            gs = gs_p.tile(
                                [128, 512], bf16, tag=f"gs{min(l, 1)}",
                                bufs=(48 if l == 0 else 12),
                            )
                            nc.scalar.activation(gs[:], g_ps[:], AF.Copy)
                            nc.vector.tensor_tensor(
                                mt[:], gs[:], ew_sl, op=ALU.mult
                            )
                        nc.tensor.matmul(
                            agg[j][:], hwn[:, qb, :], mt[:],
                            start=(qb == 0), stop=(qb == T - 1),
                        )

                if l < 2:
                    hT = hts.tile([128, N], bf16, tag="hT")
                    for j in range(NCHUNK):
                        nc.scalar.activation(
                            hT[:, j * 512:(j + 1) * 512], agg[j][:], AF.Relu
                        )
                else:
                    aggs = persist.tile([128, N], f32, tag="aggs")
                    for j in range(NCHUNK):
                        nc.scalar.activation(
                            aggs[:, j * 512:(j + 1) * 512], agg[j][:], AF.Copy
                        )
                    out_nat = persist.tile([128, T, 128], f32, tag="outn")
                    for t in range(T):
                        ps = psum.tile([128, 128], f32, tag="g", bufs=4)
                        nc.tensor.transpose(
                            ps[:], aggs[:, t * 128:(t + 1) * 128], ident_f32[:]
                        )
                        nc.scalar.activation(out_nat[:, t, :], ps[:], AF.Copy)
                    nc.sync.dma_start(
                        out_d.ap().rearrange("(t p) d -> p t d", p=128), out_nat[:]
                    )

    nc.compile()
    return nc


def kernel(**inputs):
    from concourse.bass_utils import run_bass_kernel_spmd

    x = np.asarray(inputs["x"], dtype=np.float32)
    ew = np.asarray(inputs["edge_weight"], dtype=np.float32)
    params = {}
    for l in range(3):
        params[f"W{l}"] = np.asarray(inputs[f"W{l}"], dtype=np.float32)
        params[f"b{l}"] = np.asarray(inputs[f"b{l}"], dtype=np.float32)

    nc = build_nc()
    in_maps = [
        {"x": x[c], "edge_weight": ew[c], **params} for c in range(N_CORES)
    ]
    res = run_bass_kernel_spmd(nc, in_maps, core_ids=list(range(N_CORES)))
    out = np.stack([res.results[c]["out"] for c in range(N_CORES)], axis=0)
    return out.astype(np.float32)



# revision 6
# speedup vs baseline: 1.3293x; 1.3293x over previous
"""AdaptGNN 3-layer message passing on 8 TRN2 NeuronCores.

Data-parallel over batch B=8: core c owns batch element c.

Math per core (N=2048, D=H=128):
  h = x
  for l in 0..2:
      hW  = h @ Wl + bl
      cos = normalize(hW) @ normalize(hW)^T
      h   = (ew * cos) @ hW      (+ relu except last layer)

Device-side formulation (all transposes / broadcasts hoisted to the host):
  - Host supplies xT = x^T (bf16), ewT = ew^T (bf16), W (bf16), and
    b pre-broadcast to [128,128] (f32).  Output is returned transposed
    plus the final layer's inv-norm vector; the host applies the last
    per-node scale and transposes back.
  - Loop state is hT_raw [128, N] (bf16, feature dim on partitions) with a
    deferred per-node scale s[n] from the previous layer's normalization
    (h_true[:, n] = s[n] * hT_raw[:, n]); s=1 for layer 0.
  - hW tile t: matmul(lhsT=hT_raw[:, t], W) -> psum; DVE
    scalar_tensor_tensor evac: hwn = (psum * s_col) + b_bc  (true hW, bf16).
  - Row norms: GPSIMD scalar_tensor_tensor hwn*hwn with accum_out -> n2;
    inv = 1/max(sqrt(n2), eps).
  - hWT via 16 PE tile transposes of hwn (bf16 psum, ACT evac).
  - Gram G[q,p] = hW[q]·hW[p]: matmul(lhsT=hWT qb-block, rhs=hWT j-chunk).
  - mt[q,p] = (G * inv[q]) * ewT[q,p], split across three producer paths to
    balance engines: DVE fused from psum / ACT evac + DVE mult / ACT evac +
    GPSIMD mult.
  - aggT_raw[c,p] += hwn[qb]^T @ mt  accumulated over qb in psum
    (= out[p,c] missing inv[p]; that factor is the next layer's s).
  - Grams for qb run one step ahead of aggs for qb-1 so the PE never waits
    on the mt producers.
"""

import functools

import numpy as np

N = 2048
D = 128
T = N // 128          # 16 row blocks
NCHUNK = N // 512     # 4 free-dim chunks for N=512 matmuls
N_CORES = 8
EPS = 1e-12

# mt producer assignment, cycle of 16 (qb*4+j) tiles:
#   'F' = DVE fused from psum, 'S' = ACT evac + DVE mult, 'G' = ACT evac +
#   GPSIMD mult.
MT_PATTERN = "FFGS FGFG FFGS FGFF".replace(" ", "")


@functools.lru_cache(maxsize=1)
def build_nc():
    import concourse.bass as bass
    from concourse import bacc, masks, mybir, tile

    f32 = mybir.dt.float32
    bf16 = mybir.dt.bfloat16
    AF = mybir.ActivationFunctionType
    ALU = mybir.AluOpType

    nc = bacc.Bacc(None, target_bir_lowering=False)

    xT_d = nc.declare_dram_parameter("xT", [D, N], bf16, isOutput=False)
    ewT_d = nc.declare_dram_parameter("ewT", [N, N], bf16, isOutput=False)
    w_d = []
    b_d = []
    for l in range(3):
        w_d.append(nc.declare_dram_parameter(f"W{l}", [D, D], bf16, isOutput=False))
        b_d.append(nc.declare_dram_parameter(f"B{l}", [D, D], f32, isOutput=False))
    out_d = nc.declare_dram_parameter("out", [D, N], f32, isOutput=True)
    inv_d = nc.declare_dram_parameter("inv3", [128, T], f32, isOutput=True)

    with tile.TileContext(nc) as tc:
        with (
            tc.tile_pool(name="persist", bufs=1) as persist,
            tc.tile_pool(name="consts", bufs=1) as consts,
            tc.tile_pool(name="hts", bufs=2) as hts,
            tc.tile_pool(name="hwn_p", bufs=2) as hwn_p,
            tc.tile_pool(name="hwt_p", bufs=2) as hwt_p,
            tc.tile_pool(name="inv_p", bufs=2) as inv_p,
            tc.tile_pool(name="scr_p", bufs=4) as scr_p,
            tc.tile_pool(name="gs_p", bufs=12) as gs_p,
            tc.tile_pool(name="mt_p", bufs=16) as mt_p,
            tc.tile_pool(name="psum", bufs=4, space="PSUM") as psum,
        ):
            # ---- loads (HWDGE; no casts or transposes needed) ----
            hT0 = hts.tile([128, N], bf16, tag="hT", name="hT0")
            nc.sync.dma_start(hT0[:], xT_d[:, :])
            W_bf = []
            b_bc = []
            for l in range(3):
                wb = consts.tile([128, 128], bf16, tag=f"wbf{l}", name=f"wb{l}")
                nc.sync.dma_start(wb[:], w_d[l][:, :])
                W_bf.append(wb)
                bb = consts.tile([128, 128], f32, tag=f"bbc{l}", name=f"bb{l}")
                nc.sync.dma_start(bb[:], b_d[l][:, :])
                b_bc.append(bb)

            ident_bf = consts.tile([128, 128], bf16, tag="idb")
            masks.make_identity(nc, ident_bf[:])

            # ewT slabs, in qb consumption order
            ewT = persist.tile([128, T, N], bf16, tag="ewT")
            for qb in range(T):
                nc.sync.dma_start(
                    ewT[:, qb, :], ewT_d[qb * 128:(qb + 1) * 128, :]
                )

            # ---- layers ----
            hT = hT0
            s_prev = None  # deferred per-node scale (None => 1.0)
            for l in range(3):
                hwn = hwn_p.tile([128, T, 128], bf16, tag="hwn")
                hWT = hwt_p.tile([128, N], bf16, tag="hWT")
                n2 = inv_p.tile([128, T], f32, tag="n2", name=f"n2_{l}")
                inv = inv_p.tile([128, T], f32, tag="inv", name=f"inv_{l}")

                # hW + transposes, software-pipelined so the PE isn't gated
                # on the step-2 evac latency tile by tile.
                def emit_hw(t):
                    ps = psum.tile([128, 512], f32, tag="g", bufs=4)
                    nc.tensor.matmul(
                        ps[:, 0:128], hT[:, t * 128:(t + 1) * 128], W_bf[l][:]
                    )
                    if s_prev is None:
                        nc.vector.tensor_add(hwn[:, t, :], ps[:, 0:128], b_bc[l][:])
                    else:
                        nc.vector.scalar_tensor_tensor(
                            hwn[:, t, :], ps[:, 0:128], s_prev[:, t:t + 1],
                            b_bc[l][:], op0=ALU.mult, op1=ALU.add,
                        )
                    sq = scr_p.tile([128, 128], bf16, tag="sq", bufs=4)
                    nc.scalar.activation(
                        sq[:], hwn[:, t, :], AF.Square,
                        accum_out=n2[:, t:t + 1],
                    )

                def emit_tr(t):
                    ps2 = psum.tile([128, 512], bf16, tag="g", bufs=4)
                    nc.tensor.transpose(ps2[:, 0:128], hwn[:, t, :], ident_bf[:])
                    dst = hWT[:, t * 128:(t + 1) * 128]
                    if t % 2 == 0:
                        nc.scalar.activation(dst, ps2[:, 0:128], AF.Copy)
                    else:
                        nc.vector.tensor_copy(dst, ps2[:, 0:128])

                for t in range(4):
                    emit_hw(t)
                for t in range(4, T):
                    emit_hw(t)
                    emit_tr(t - 4)
                for t in range(T - 4, T):
                    emit_tr(t)

                # inv = 1 / max(sqrt(n2), eps)
                nrm = inv_p.tile([128, T], f32, tag="nrm", name=f"nrm_{l}")
                nc.scalar.activation(nrm[:], n2[:], AF.Sqrt)
                nc.vector.tensor_scalar_max(nrm[:], nrm[:], EPS)
                nc.vector.reciprocal(inv[:], nrm[:])

                # main loop: grams for qb one step ahead of aggs for qb-1
                agg = [
                    psum.tile([128, 512], f32, tag="agg", name=f"agg{l}_{j}")
                    for j in range(NCHUNK)
                ]

                def emit_gram(qb):
                    tiles = []
                    for j in range(NCHUNK):
                        g_ps = psum.tile([128, 512], f32, tag="g", bufs=4)
                        nc.tensor.matmul(
                            g_ps[:],
                            hWT[:, qb * 128:(qb + 1) * 128],
                            hWT[:, j * 512:(j + 1) * 512],
                        )
                        tiles.append(g_ps)
                    return tiles

                def emit_mt_agg(qb, g_tiles):
                    for j in range(NCHUNK):
                        g_ps = g_tiles[j]
                        ew_sl = ewT[:, qb, j * 512:(j + 1) * 512]
                        mt = mt_p.tile([128, 512], bf16, tag="mt")
                        path = MT_PATTERN[(qb * NCHUNK + j) % len(MT_PATTERN)]
                        if path == "F":
                            nc.vector.scalar_tensor_tensor(
                                mt[:], g_ps[:], inv[:, qb:qb + 1], ew_sl,
                                op0=ALU.mult, op1=ALU.mult,
                            )
                        else:
                            gs = gs_p.tile([128, 512], bf16, tag="gs")
                            nc.scalar.activation(
                                gs[:], g_ps[:], AF.Copy, scale=inv[:, qb:qb + 1]
                            )
                            eng = nc.vector if path == "S" else nc.gpsimd
                            eng.tensor_tensor(
                                mt[:], gs[:], ew_sl, op=ALU.mult
                            )
                        nc.tensor.matmul(
                            agg[j][:], hwn[:, qb, :], mt[:],
                            start=(qb == 0), stop=(qb == T - 1),
                        )

                prev = emit_gram(0)
                for qb in range(1, T):
                    cur = emit_gram(qb)
                    emit_mt_agg(qb - 1, prev)
                    prev = cur
                emit_mt_agg(T - 1, prev)

                if l < 2:
                    hT = hts.tile([128, N], bf16, tag="hT", name=f"hT{l + 1}")
                    for j in range(NCHUNK):
                        nc.scalar.activation(
                            hT[:, j * 512:(j + 1) * 512], agg[j][:], AF.Relu
                        )
                    s_prev = inv
                else:
                    aggs = persist.tile([128, N], f32, tag="aggs")
                    for j in range(NCHUNK):
                        nc.scalar.activation(
                            aggs[:, j * 512:(j + 1) * 512], agg[j][:], AF.Copy
                        )
                    nc.sync.dma_start(out_d[:, :], aggs[:])
                    nc.sync.dma_start(inv_d[:, :], inv[:])

    nc.compile()
    return nc


def prepare_in_maps(x, ew, params):
    """Host-side input transform: transposes, casts, bias broadcast."""
    import ml_dtypes

    bf16 = ml_dtypes.bfloat16
    common = {}
    for l in range(3):
        common[f"W{l}"] = np.ascontiguousarray(
            params[f"W{l}"].astype(bf16)
        )
        common[f"B{l}"] = np.ascontiguousarray(
            np.broadcast_to(
                params[f"b{l}"].astype(np.float32)[None, :], (128, 128)
            )
        )
    in_maps = []
    for c in range(N_CORES):
        in_maps.append({
            "xT": np.ascontiguousarray(x[c].T.astype(bf16)),
            "ewT": np.ascontiguousarray(ew[c].T.astype(bf16)),
            **common,
        })
    return in_maps


def assemble_output(results):
    """Host-side output transform: apply deferred inv scale, transpose."""
    out = np.empty((N_CORES, N, D), dtype=np.float32)
    for c in range(N_CORES):
        aggT = results[c]["out"]          # [D, N], missing inv3[n] scale
        inv3 = results[c]["inv3"]         # [128, T]; n = t*128 + p
        s = inv3.T.reshape(N)             # s[n]
        out[c] = aggT.T * s[:, None]
    return out


def kernel(**inputs):
    from concourse.bass_utils import run_bass_kernel_spmd

    x = np.asarray(inputs["x"], dtype=np.float32)
    ew = np.asarray(inputs["edge_weight"], dtype=np.float32)
    params = {}
    for l in range(3):
        params[f"W{l}"] = np.asarray(inputs[f"W{l}"], dtype=np.float32)
        params[f"b{l}"] = np.asarray(inputs[f"b{l}"], dtype=np.float32)

    nc = build_nc()
    in_maps = prepare_in_maps(x, ew, params)
    res = run_bass_kernel_spmd(nc, in_maps, core_ids=list(range(N_CORES)))
    return assemble_output(res.results)
